# revision 3
# baseline (speedup 1.0000x reference)
"""DySepConvAtten Trainium2 kernel.

out = LayerNorm( pw @ relu(depthwise_conv1d(value, dw)) ), where
[dw | pw] = query @ W_wl + b_wl  per (batch, position).

Sharding: pure data parallelism, B=512 split over 8 NeuronCores (64 each).
"""

import numpy as np

B, N, C, K = 512, 100, 256, 3
NCORES = 8
NB = B // NCORES          # batches per core
SLAB = 4                  # batches per DMA slab
NSLAB = NB // SLAB
LN_EPS = 1e-5

_cache: dict = {}


def _build(apply_affine: bool, nb: int):
    import concourse.bass as bass
    import concourse.tile as tile
    from concourse import bacc, mybir

    fp32 = mybir.dt.float32
    AF = mybir.ActivationFunctionType
    OP = mybir.AluOpType

    nc = bacc.Bacc("TRN2", target_bir_lowering=False, debug=False)

    nslab = nb // SLAB

    # DRAM tensors (per-core shapes)
    qT_d = nc.dram_tensor("qT", (nb, 128, 2 * N), fp32, kind="ExternalInput")
    v_d = nc.dram_tensor("v", (nb, N, C), fp32, kind="ExternalInput")
    w2_d = nc.dram_tensor("w2", (128, 2 * (N + K)), fp32, kind="ExternalInput")
    bpw_d = nc.dram_tensor("bpw", (N, 1), fp32, kind="ExternalInput")
    bdw_d = nc.dram_tensor("bdw", (N, K), fp32, kind="ExternalInput")
    if apply_affine:
        gam_d = nc.dram_tensor("gam", (N, C), fp32, kind="ExternalInput")
        bet_d = nc.dram_tensor("bet", (N, C), fp32, kind="ExternalInput")
    out_d = nc.dram_tensor("out", (nb, N, C), fp32, kind="ExternalOutput")

    NK = N + K

    with tile.TileContext(nc) as tc:
        with (
            tc.tile_pool(name="const", bufs=1) as cpool,
            tc.tile_pool(name="slab_in", bufs=2) as sin_pool,
            tc.tile_pool(name="slab_out", bufs=2) as sout_pool,
            tc.tile_pool(name="work", bufs=3) as wpool,
            tc.tile_pool(name="small", bufs=4) as spool,
            tc.tile_pool(name="ps_dw", bufs=2, space="PSUM") as ps_dw_pool,
            tc.tile_pool(name="ps_pwT", bufs=2, space="PSUM") as ps_pwT_pool,
            tc.tile_pool(name="ps_out", bufs=2, space="PSUM") as ps_out_pool,
        ):
            w2_t = cpool.tile([128, 2 * NK], fp32)
            nc.sync.dma_start(w2_t[:], w2_d.ap()[:])
            bpw_t = cpool.tile([N, 1], fp32)
            nc.sync.dma_start(bpw_t[:], bpw_d.ap()[:])
            bdw_t = cpool.tile([N, K], fp32)
            nc.sync.dma_start(bdw_t[:], bdw_d.ap()[:])
            eps_t = cpool.tile([N, 1], fp32)
            nc.gpsimd.memset(eps_t[:], LN_EPS)
            if apply_affine:
                gam_t = cpool.tile([N, C], fp32)
                nc.sync.dma_start(gam_t[:], gam_d.ap()[:])
                bet_t = cpool.tile([N, C], fp32)
                nc.sync.dma_start(bet_t[:], bet_d.ap()[:])

            for s in range(nslab):
                # ---- slab loads ----
                qT_s = sin_pool.tile([128, SLAB, 2 * N], fp32, tag="qT_s")
                nc.sync.dma_start(
                    qT_s[:],
                    qT_d.ap()[s * SLAB:(s + 1) * SLAB].rearrange("b p j -> p b j"),
                )
                vp_s = sin_pool.tile([N, SLAB, C + 2], fp32, tag="vp_s")
                # zero the two pad columns of each batch block
                nc.gpsimd.memset(vp_s[:, :, 0:(C + 2):(C + 1)], 0.0)
                nc.sync.dma_start(
                    vp_s[:, :, 1:(C + 1)],
                    v_d.ap()[s * SLAB:(s + 1) * SLAB].rearrange("b n c -> n b c"),
                )
                out_s = sout_pool.tile([N, SLAB, C], fp32, tag="out_s")

                for j in range(SLAB):
                    qT0 = qT_s[:, j, 0:N]
                    qT1 = qT_s[:, j, N:2 * N]
                    vp = vp_s[:, j, :]

                    # pwT[m, n] = pw[n, m] = dy[n, 3+m]
                    ps_pwT = ps_pwT_pool.tile([N, N], fp32, tag="ps_pwT")
                    nc.tensor.matmul(ps_pwT[:], w2_t[:, K:NK], qT0, start=True, stop=False)
                    nc.tensor.matmul(ps_pwT[:], w2_t[:, NK + K:2 * NK], qT1, start=False, stop=True)
                    pwT_sb = wpool.tile([N, N], fp32, tag="pwT_sb")
                    # add per-partition bias b_wl[3+m]
                    nc.scalar.activation(pwT_sb[:], ps_pwT[:], AF.Identity, bias=bpw_t[:])

                    # dw[n, k] = dy[n, k], natural layout
                    ps_dw = ps_dw_pool.tile([N, K], fp32, tag="ps_dw")
                    nc.tensor.matmul(ps_dw[:], qT0, w2_t[:, 0:K], start=True, stop=False)
                    nc.tensor.matmul(ps_dw[:], qT1, w2_t[:, NK:NK + K], start=False, stop=True)
                    dw_sb = spool.tile([N, K], fp32, tag="dw_sb")
                    nc.vector.tensor_add(dw_sb[:], ps_dw[:], bdw_t[:])

                    # depthwise conv along C + relu
                    t0 = wpool.tile([N, C], fp32, tag="t0")
                    nc.vector.tensor_scalar_mul(t0[:], vp[:, 0:C], dw_sb[:, 0:1])
                    acc = wpool.tile([N, C], fp32, tag="acc")
                    nc.vector.scalar_tensor_tensor(
                        acc[:], vp[:, 1:C + 1], dw_sb[:, 1:2], t0[:], op0=OP.mult, op1=OP.add)
                    acc2 = wpool.tile([N, C], fp32, tag="acc2")
                    nc.vector.scalar_tensor_tensor(
                        acc2[:], vp[:, 2:C + 2], dw_sb[:, 2:3], acc[:], op0=OP.mult, op1=OP.add)
                    depth = wpool.tile([N, C], fp32, tag="depth")
                    nc.scalar.activation(depth[:], acc2[:], AF.Relu)

                    # pointwise: out = pw @ depth
                    ps_out = ps_out_pool.tile([N, C], fp32, tag="ps_out")
                    nc.tensor.matmul(ps_out[:], pwT_sb[:], depth[:], start=True, stop=True)

                    # LayerNorm over C
                    stats = spool.tile([N, 6], fp32, tag="stats")
                    nc.vector.bn_stats(stats[:], ps_out[:])
                    mv = spool.tile([N, 2], fp32, tag="mv")
                    nc.vector.bn_aggr(mv[:], stats[:])
                    std = spool.tile([N, 1], fp32, tag="std")
                    nc.scalar.activation(std[:], mv[:, 1:2], AF.Sqrt, bias=eps_t[:])
                    rs = spool.tile([N, 1], fp32, tag="rs")
                    nc.vector.reciprocal(rs[:], std[:])
                    nmr = spool.tile([N, 1], fp32, tag="nmr")
                    # nmr = -mean * rs
                    nc.vector.tensor_scalar(
                        nmr[:], mv[:, 0:1], rs[:], -1.0, op0=OP.mult, op1=OP.mult)
                    if apply_affine:
                        nrm = wpool.tile([N, C], fp32, tag="nrm")
                        nc.scalar.activation(
                            nrm[:], ps_out[:], AF.Identity, bias=nmr[:], scale=rs[:])
                        tmp = wpool.tile([N, C], fp32, tag="tmp")
                        nc.vector.tensor_mul(tmp[:], nrm[:], gam_t[:])
                        nc.vector.tensor_add(out_s[:, j, :], tmp[:], bet_t[:])
                    else:
                        nc.scalar.activation(
                            out_s[:, j, :], ps_out[:], AF.Identity, bias=nmr[:], scale=rs[:])

                nc.sync.dma_start(
                    out_d.ap()[s * SLAB:(s + 1) * SLAB].rearrange("b n c -> n b c"),
                    out_s[:],
                )

    nc.compile()
    return nc


def _get_nc(apply_affine: bool, nb: int):
    key = (apply_affine, nb)
    if key not in _cache:
        _cache[key] = _build(apply_affine, nb)
    return _cache[key]


def _host_prep(query, value, W_wl, b_wl, ln_gamma, ln_beta, n_cores=NCORES):
    """Build per-core input maps (numpy only)."""
    Bf = query.shape[0]
    nb = Bf // n_cores
    apply_affine = not (
        np.all(ln_gamma == np.float32(1.0)) and np.all(ln_beta == np.float32(0.0))
    )
    # qT[b] : [128, 2*N] with qT[b][p, j*N + n] = query[b, n, 128*j + p]
    qT = np.ascontiguousarray(
        query.transpose(0, 2, 1)          # [B, C, N]
        .reshape(Bf, 2, 128, N)
        .transpose(0, 2, 1, 3)            # [B, 128, 2, N]
        .reshape(Bf, 128, 2 * N)
    ).astype(np.float32)
    # w2 : [128, 2*(N+K)] with w2[p, j*(N+K) + k] = W_wl[128*j + p, k]
    w2 = np.ascontiguousarray(
        W_wl.reshape(2, 128, N + K).transpose(1, 0, 2).reshape(128, 2 * (N + K))
    ).astype(np.float32)
    bpw = np.ascontiguousarray(b_wl[K:].reshape(N, 1)).astype(np.float32)
    bdw = np.ascontiguousarray(np.broadcast_to(b_wl[:K], (N, K))).astype(np.float32)

    in_maps = []
    for c in range(n_cores):
        m = {
            "qT": qT[c * nb:(c + 1) * nb],
            "v": np.ascontiguousarray(value[c * nb:(c + 1) * nb]).astype(np.float32),
            "w2": w2,
            "bpw": bpw,
            "bdw": bdw,
        }
        if apply_affine:
            m["gam"] = np.ascontiguousarray(
                np.broadcast_to(ln_gamma, (N, C))).astype(np.float32)
            m["bet"] = np.ascontiguousarray(
                np.broadcast_to(ln_beta, (N, C))).astype(np.float32)
        in_maps.append(m)
    return in_maps, apply_affine, nb


def kernel(query, value, W_wl, b_wl, ln_gamma, ln_beta):
    from concourse import bass_utils

    in_maps, apply_affine, nb = _host_prep(
        query, value, W_wl, b_wl, ln_gamma, ln_beta)
    nc = _get_nc(apply_affine, nb)
    res = bass_utils.run_bass_kernel_spmd(
        nc, in_maps, core_ids=list(range(NCORES)))
    out = np.concatenate([res.results[c]["out"] for c in range(NCORES)], axis=0)
    return out.astype(np.float32)


# revision 8
# speedup vs baseline: 1.1601x; 1.1601x over previous
"""DySepConvAtten Trainium2 kernel.

out = LayerNorm( pw @ relu(depthwise_conv1d(value, dw)) ), where
[dw | pw] = query @ W_wl + b_wl  per (batch, position).

Sharding: pure data parallelism, B=512 split over 8 NeuronCores (64 each).
"""

import numpy as np
import ml_dtypes

B, N, C, K = 512, 100, 256, 3
NCORES = 8
NB = B // NCORES          # batches per core
SLAB = 4                  # batches per DMA slab / matmul batch-group
LN_EPS = 1e-5

# config
CFG = dict(bf16_conv=False, bf16_q=False, fp32r=True)

_cache: dict = {}


def _build(apply_affine: bool, nb: int, cfg=CFG):
    import concourse.bass as bass
    import concourse.tile as tile
    from concourse import bacc, mybir

    fp32 = mybir.dt.float32
    bf16 = mybir.dt.bfloat16
    AF = mybir.ActivationFunctionType
    OP = mybir.AluOpType

    fp32r = mybir.dt.float32r
    # matmul-operand dtype: fp32r runs the PE at full rate (1 cyc/row for
    # moving free >= 256) vs 4 cyc/row for plain fp32
    mdt = fp32r if cfg["fp32r"] else fp32
    cdt = bf16 if cfg["bf16_conv"] else fp32    # conv path dtype
    qdt = bf16 if cfg["bf16_q"] else mdt        # q/W matmul dtype
    ddt = bf16 if cfg["bf16_conv"] else mdt     # depth (pointwise rhs) dtype
    def mmv(ap):
        return ap

    nc = bacc.Bacc("TRN2", target_bir_lowering=False, debug=False)

    nslab = nb // SLAB
    NK = N + K

    qT_d = nc.dram_tensor("qT", (nslab, 128, SLAB, 2 * N), qdt, kind="ExternalInput")
    v_d = nc.dram_tensor("v", (nslab, N, SLAB, C + 2), cdt, kind="ExternalInput")
    w2_d = nc.dram_tensor("w2", (128, 2 * NK), qdt, kind="ExternalInput")
    bpw_d = nc.dram_tensor("bpw", (N, 1), fp32, kind="ExternalInput")
    bdw_d = nc.dram_tensor("bdw", (K, 1), fp32, kind="ExternalInput")
    id3_d = nc.dram_tensor("id3", (K, K), fp32, kind="ExternalInput")
    if apply_affine:
        gam_d = nc.dram_tensor("gam", (N, C), fp32, kind="ExternalInput")
        bet_d = nc.dram_tensor("bet", (N, C), fp32, kind="ExternalInput")
    out_d = nc.dram_tensor("out", (nslab, N, SLAB, C), fp32, kind="ExternalOutput")

    with tile.TileContext(nc) as tc:
        with (
            tc.tile_pool(name="const", bufs=1) as cpool,
            tc.tile_pool(name="slab_in", bufs=3) as sin_pool,
            tc.tile_pool(name="slab_out", bufs=2) as sout_pool,
            tc.tile_pool(name="work", bufs=3) as wpool,
            tc.tile_pool(name="small", bufs=6) as spool,
            tc.tile_pool(name="ps_dw", bufs=2, space="PSUM") as ps_dw_pool,
            tc.tile_pool(name="ps_pwT", bufs=2, space="PSUM") as ps_pwT_pool,
            tc.tile_pool(name="ps_dwT", bufs=2, space="PSUM") as ps_dwT_pool,
            tc.tile_pool(name="ps_out", bufs=2, space="PSUM") as ps_out_pool,
        ):
            w2_t = cpool.tile([128, 2 * NK], qdt)
            nc.sync.dma_start(w2_t[:], w2_d.ap()[:])
            bpw_t = cpool.tile([N, 1], fp32)
            nc.sync.dma_start(bpw_t[:], bpw_d.ap()[:])
            bdw_t = cpool.tile([K, 1], fp32)
            nc.sync.dma_start(bdw_t[:], bdw_d.ap()[:])
            id3_t = cpool.tile([K, K], fp32)
            nc.sync.dma_start(id3_t[:], id3_d.ap()[:])
            eps_t = cpool.tile([N, 1], fp32)
            nc.gpsimd.memset(eps_t[:], LN_EPS)
            if apply_affine:
                gam_t = cpool.tile([N, C], fp32)
                nc.sync.dma_start(gam_t[:], gam_d.ap()[:])
                bet_t = cpool.tile([N, C], fp32)
                nc.sync.dma_start(bet_t[:], bet_d.ap()[:])

            # W column layout: w2[:, j*NK + k], k in [0,K) = dw cols, [K,NK) = pw cols
            for s in range(nslab):
                # ---- slab loads (contiguous per partition) ----
                qT_s = sin_pool.tile([128, SLAB, 2 * N], qdt, tag="qT_s")
                nc.sync.dma_start(qT_s[:], qT_d.ap()[s])
                vp_s = sin_pool.tile([N, SLAB, C + 2], cdt, tag="vp_s")
                nc.sync.dma_start(vp_s[:], v_d.ap()[s])
                out_s = sout_pool.tile([N, SLAB, C], fp32, tag="out_s")

                # ---- batched dy matmuls over the whole slab ----
                # pwT_s[m, j*N+n] = pw[b_j, n, m];  moving = [128,(SLAB,N)] (400 cols)
                ps_pwT = ps_pwT_pool.tile([N, SLAB * N], fp32, tag="ps_pwT")
                nc.tensor.matmul(ps_pwT[:], mmv(w2_t[:, K:NK]),
                                 mmv(qT_s[:, :, 0:N]), start=True, stop=False)
                nc.tensor.matmul(ps_pwT[:], mmv(w2_t[:, NK + K:2 * NK]),
                                 mmv(qT_s[:, :, N:2 * N]), start=False, stop=True)
                pwT_sb = wpool.tile([N, SLAB * N], ddt, tag="pwT_sb")
                nc.scalar.activation(pwT_sb[:], ps_pwT[:], AF.Identity, bias=bpw_t[:])

                # dwT_s[k, j*N+n] = dw[b_j, n, k]
                ps_dwT = ps_dwT_pool.tile([K, SLAB * N], fp32, tag="ps_dwT")
                nc.tensor.matmul(ps_dwT[:], mmv(w2_t[:, 0:K]),
                                 mmv(qT_s[:, :, 0:N]), start=True, stop=False)
                nc.tensor.matmul(ps_dwT[:], mmv(w2_t[:, NK:NK + K]),
                                 mmv(qT_s[:, :, N:2 * N]), start=False, stop=True)
                dwT_sb = spool.tile([K, SLAB * N], fp32, tag="dwT_sb")
                nc.scalar.activation(dwT_sb[:], ps_dwT[:], AF.Identity, bias=bdw_t[:])

                acc2_s = wpool.tile([N, SLAB, C], cdt, tag="acc2_s")

                for j in range(SLAB):
                    vp = vp_s[:, j, :]
                    # dw[n, k] for this batch via PE transpose of dwT
                    ps_dw = ps_dw_pool.tile([N, K], fp32, tag="ps_dw")
                    nc.tensor.transpose(ps_dw[:], dwT_sb[:, j * N:(j + 1) * N], id3_t[:])
                    dw_sb = spool.tile([N, K], cdt, tag="dw_sb")
                    nc.scalar.copy(dw_sb[:], ps_dw[:])

                    # depthwise conv along C (relu deferred to slab level)
                    t0 = wpool.tile([N, C], cdt, tag="t0")
                    nc.vector.tensor_scalar_mul(t0[:], vp[:, 0:C], dw_sb[:, 0:1])
                    acc = wpool.tile([N, C], cdt, tag="acc")
                    nc.vector.scalar_tensor_tensor(
                        acc[:], vp[:, 1:C + 1], dw_sb[:, 1:2], t0[:],
                        op0=OP.mult, op1=OP.add)
                    nc.vector.scalar_tensor_tensor(
                        acc2_s[:, j, :], vp[:, 2:C + 2], dw_sb[:, 2:3], acc[:],
                        op0=OP.mult, op1=OP.add)

                # relu over whole slab, cast to fp32 for the pointwise matmul
                depth_s = wpool.tile([N, SLAB, C], ddt, tag="depth_s")
                nc.scalar.activation(depth_s[:], acc2_s[:], AF.Relu)

                mv_s = spool.tile([N, SLAB, 2], fp32, tag="mv_s")
                ps_out_pairs = []
                for j in range(SLAB):
                    p, i = divmod(j, 2)
                    if i == 0:
                        ps_out = ps_out_pool.tile([N, 2, C], fp32, tag="ps_out")
                        ps_out_pairs.append(ps_out)
                    ps_out = ps_out_pairs[p]
                    # pointwise: out = pw @ depth
                    nc.tensor.matmul(ps_out[:, i, :],
                                     mmv(pwT_sb[:, j * N:(j + 1) * N]),
                                     mmv(depth_s[:, j, :]), start=True, stop=True)
                    # LayerNorm stats (single group; multi-group bn_stats
                    # breaks when the AP optimizer flattens contiguous dims)
                    stats = spool.tile([N, 6], fp32, tag="stats")
                    nc.vector.bn_stats(stats[:], ps_out[:, i, :])
                    nc.vector.bn_aggr(mv_s[:, j, :], stats[:])

                # batched sqrt(var+eps) / recip / -mu*rs over the slab
                std_s = spool.tile([N, SLAB], fp32, tag="std_s")
                nc.scalar.activation(std_s[:], mv_s[:, :, 1], AF.Sqrt, bias=eps_t[:])
                rs_s = spool.tile([N, SLAB], fp32, tag="rs_s")
                nc.vector.reciprocal(rs_s[:], std_s[:])
                nmr_s = spool.tile([N, SLAB], fp32, tag="nmr_s")
                nc.vector.scalar_tensor_tensor(
                    nmr_s[:], mv_s[:, :, 0], -1.0, rs_s[:], op0=OP.mult, op1=OP.mult)

                for j in range(SLAB):
                    p, i = divmod(j, 2)
                    ps_out = ps_out_pairs[p]
                    if apply_affine:
                        nrm = wpool.tile([N, C], fp32, tag="nrm")
                        nc.scalar.activation(
                            nrm[:], ps_out[:, i, :], AF.Identity,
                            bias=nmr_s[:, j:j + 1], scale=rs_s[:, j:j + 1])
                        tmp = wpool.tile([N, C], fp32, tag="tmp")
                        nc.vector.tensor_mul(tmp[:], nrm[:], gam_t[:])
                        nc.vector.tensor_add(out_s[:, j, :], tmp[:], bet_t[:])
                    else:
                        nc.scalar.activation(
                            out_s[:, j, :], ps_out[:, i, :], AF.Identity,
                            bias=nmr_s[:, j:j + 1], scale=rs_s[:, j:j + 1])

                nc.sync.dma_start(out_d.ap()[s], out_s[:])

    nc.compile()
    return nc


def _get_nc(apply_affine: bool, nb: int):
    key = (apply_affine, nb, tuple(sorted(CFG.items())))
    if key not in _cache:
        _cache[key] = _build(apply_affine, nb, CFG)
    return _cache[key]


def _host_prep(query, value, W_wl, b_wl, ln_gamma, ln_beta, n_cores=NCORES, cfg=CFG):
    """Build per-core input maps (numpy only)."""
    Bf = query.shape[0]
    nb = Bf // n_cores
    nslab = nb // SLAB
    apply_affine = not (
        np.all(ln_gamma == np.float32(1.0)) and np.all(ln_beta == np.float32(0.0))
    )
    qnp = ml_dtypes.bfloat16 if cfg["bf16_q"] else np.float32
    cnp = ml_dtypes.bfloat16 if cfg["bf16_conv"] else np.float32

    # qT[b] : [128, 2*N] with qT[b][p, j*N + n] = query[b, n, 128*j + p]
    qT = (
        query.transpose(0, 2, 1)          # [B, C, N]
        .reshape(Bf, 2, 128, N)
        .transpose(0, 2, 1, 3)            # [B, 128, 2, N]
        .reshape(Bf, 128, 2 * N)
    )
    # slab layout: [B//SLAB, 128, SLAB, 2N]
    qTs = np.ascontiguousarray(
        qT.reshape(Bf // SLAB, SLAB, 128, 2 * N).transpose(0, 2, 1, 3)
    ).astype(qnp)

    # value: pad C with one zero col each side -> [B//SLAB, N, SLAB, C+2]
    vp = np.zeros((Bf, N, C + 2), np.float32)
    vp[:, :, 1:C + 1] = value
    vps = np.ascontiguousarray(
        vp.reshape(Bf // SLAB, SLAB, N, C + 2).transpose(0, 2, 1, 3)
    ).astype(cnp)

    # w2 : [128, 2*(N+K)] with w2[p, j*(N+K) + k] = W_wl[128*j + p, k]
    w2 = np.ascontiguousarray(
        W_wl.reshape(2, 128, N + K).transpose(1, 0, 2).reshape(128, 2 * (N + K))
    ).astype(qnp)
    bpw = np.ascontiguousarray(b_wl[K:].reshape(N, 1)).astype(np.float32)
    bdw = np.ascontiguousarray(b_wl[:K].reshape(K, 1)).astype(np.float32)
    id3 = np.eye(K, dtype=np.float32)

    spc = nslab  # slabs per core
    in_maps = []
    for c in range(n_cores):
        m = {
            "qT": qTs[c * spc:(c + 1) * spc],
            "v": vps[c * spc:(c + 1) * spc],
            "w2": w2,
            "bpw": bpw,
            "bdw": bdw,
            "id3": id3,
        }
        if apply_affine:
            m["gam"] = np.ascontiguousarray(
                np.broadcast_to(ln_gamma, (N, C))).astype(np.float32)
            m["bet"] = np.ascontiguousarray(
                np.broadcast_to(ln_beta, (N, C))).astype(np.float32)
        in_maps.append(m)
    return in_maps, apply_affine, nb


def _gather(results, n_cores, nb):
    # out dram is [nslab, N, SLAB, C] per core -> [B, N, C]
    outs = []
    for c in range(n_cores):
        o = results[c]["out"]                      # [nslab, N, SLAB, C]
        o = o.transpose(0, 2, 1, 3).reshape(nb, N, C)
        outs.append(o)
    return np.concatenate(outs, axis=0)


def kernel(query, value, W_wl, b_wl, ln_gamma, ln_beta):
    from concourse import bass_utils

    in_maps, apply_affine, nb = _host_prep(
        query, value, W_wl, b_wl, ln_gamma, ln_beta)
    nc = _get_nc(apply_affine, nb)
    res = bass_utils.run_bass_kernel_spmd(
        nc, in_maps, core_ids=list(range(NCORES)))
    return np.ascontiguousarray(_gather(res.results, NCORES, nb)).astype(np.float32)
